# revision 4
# baseline (speedup 1.0000x reference)
"""Trainium2 Bass kernel for nn_DigitLayer (CapsNet digit-capsule layer).

Math note: the reference's routing softmax acts on a size-1 axis, so the
coupling coefficients are exactly 1.0 on every iteration and the whole
3-iteration routing collapses to

    S[b,d,i] = sum_{p,j} W[p,d,i,j] * x[b,p,j]
    out      = squash(S)  over i (the 16-dim)

i.e. one [B, P*8] @ [P*8, D*16] matmul plus a per-(b,d) squash.

Distribution: the contraction dim P (1152) is sharded across the 8 cores so
every byte of x and W is read from HBM exactly once chip-wide. Each core
computes a partial S[b, (d,i)] over its P-shard for all 256 batches; the host
sums the 8 partial tensors and applies the (collapsed-routing) squash.

Schedule (v2, pipelined): the per-core contraction (KL=1152 = 9 chunks of
128) is split into 5 slabs of (2,2,2,2,1) chunks. Each slab packs its x and
W chunk data into ONE DRAM tensor with fat 128-partition rows, loaded by one
HWDGE DMA on the scalar (ACT) ring; slabs stream FIFO and the PE consumes
them as they land (2 accumulating matmuls per chunk, one per batch half /
PSUM bank) instead of waiting for the full load. Trace evidence from v1:
the two HWDGE rings' descriptor generation is effectively serialized and
flow-controlled to SDMA consumption, so all input goes on one ring (the
scalar engine also exits the framework preamble ~0.7us before sync, so its
kick fires earliest) and the tail store gets the other. The tail is
fp32->fp16 copies of the two PSUM banks run in parallel on DVE and ACT,
then a single f16 store (halved bytes) kicked from the idle sync ring.

Inputs are fed to the device as float16 (fp8 was measured at 4-6e-2 rel err
vs the 2e-2 gate -- rejected); accumulation is fp32 in PSUM, and the f16
partial-sum store adds ~1e-4 relative error, well within budget.

Device-side layout (per core, all host-prepped, SBUF-native):
    xw{s} [128, n_s*416] f16 : slab s; per partition row = n_s x-chunks
                               (256 f16 each, batch-major) then n_s w-chunks
                               (160 f16 each), k_local = c*128 + partition
                               = p_local*8 + j, n = d*16 + i
    out  [128, 2, 160] f16   : partial S, out[p, m, n] = S[m*128+p, n]
"""

import numpy as np

import concourse.bacc as bacc
import concourse.mybir as mybir
from concourse.bass_utils import run_bass_kernel_spmd

B, P, D, VP, VD = 256, 1152, 10, 8, 16
NCORES = 8
PL = P // NCORES           # 144 primary capsules per core
KL = PL * VP               # 1152 local contraction length
KCH = KL // 128            # 9 k-chunks of 128
N_OUT = D * VD             # 160
MB = 128                   # batch chunk (matmul M / PSUM partitions)
NMB = B // MB              # 2

SLABS = [2, 2, 2, 2, 1]    # chunks per slab (sum == KCH); last slab small
                           # so the PE tail after the final DMA byte is short
assert sum(SLABS) == KCH
SLAB_OFF = [sum(SLABS[:s]) for s in range(len(SLABS))]

_cache = {}


def _hoist_first(nc, instrs):
    """Move the given instructions to the front of their engine's stream so
    the input DMAs issue before the framework preamble (all-engine barrier)
    and their transfer latency overlaps it."""
    names = {i.name for i in instrs}
    for bb in nc.main_func.blocks:
        if not any(ins.name in names for ins in bb.instructions):
            continue
        by_engine = {}
        for ins in bb.instructions:
            if ins.name in names:
                by_engine.setdefault(ins.engine, []).append(ins)
        new = []
        emitted = set()
        for ins in bb.instructions:
            if ins.name in names:
                continue
            e = ins.engine
            if e in by_engine and e not in emitted:
                new.extend(by_engine[e])
                emitted.add(e)
            new.append(ins)
        for e, lst in by_engine.items():
            if e not in emitted:
                new.extend(lst)
        bb.instructions[:] = new


def _strip_const_memsets(nc):
    """Drop the framework's const-AP Memsets (unused by this kernel) from the
    Pool stream. Nothing reads those SBUF constants here, and removing them
    leaves the PE's first LDWEIGHTS/MATMUL as the kernel's first compute
    instruction."""
    removed = 0
    for bb in nc.main_func.blocks:
        keep = [
            i for i in bb.instructions
            if not (type(i).__name__ == "InstMemset"
                    and "const-" in str(getattr(i, "outs", "")))
        ]
        removed += len(bb.instructions) - len(keep)
        bb.instructions[:] = keep
    return removed


def _build():
    """Raw-bass kernel (no TileContext), hand-placed semaphores.

    Hard-won rules baked in here:
      * One semaphore per DMA: a HWDGE DMA completes as 16 unordered +1
        sub-increments, so intermediate thresholds on a shared sem race.
      * The PE gate must wait on the DMA completion semaphores; an engine
        DRAIN does NOT barrier HWDGE DMA data (cold-run NaNs).
      * The final stop-matmul's then_inc covers bank 0; bank 1 gets a full
        PE drain before its copy.
      * No wait on the output DMA semaphore: the runtime end-of-program
        barrier covers it.
    """
    dt_in = mybir.dt.float16
    nc = bacc.Bacc("TRN2", debug=False, num_devices=NCORES)
    xw = [
        nc.dram_tensor(f"xw{s}", [128, n * (B + N_OUT)], dt_in,
                       kind="ExternalInput").ap()
        for s, n in enumerate(SLABS)
    ]
    out = nc.dram_tensor("out", [128, NMB, N_OUT], dt_in,
                         kind="ExternalOutput").ap()

    from contextlib import ExitStack
    with ExitStack() as ctx:
        sbs = [
            ctx.enter_context(nc.sbuf_tensor(f"sb{s}", [128, n * (B + N_OUT)], dt_in))
            for s, n in enumerate(SLABS)
        ]
        pts = [
            ctx.enter_context(nc.psum_tensor(f"pt{m}", [MB, N_OUT], mybir.dt.float32))
            for m in range(NMB)
        ]
        osb = ctx.enter_context(nc.sbuf_tensor("osb", [MB, NMB, N_OUT], dt_in))
        sem_in = [
            ctx.enter_context(nc.semaphore(name=f"sem_in{s}"))
            for s in range(len(SLABS))
        ]
        sem_d = ctx.enter_context(nc.semaphore(name="sem_d"))
        sem_cp = ctx.enter_context(nc.semaphore(name="sem_cp"))
        sem_out = ctx.enter_context(nc.semaphore(name="sem_out"))

        # input slab DMAs: all on the scalar (ACT) HWDGE ring, FIFO order
        in_dmas = [
            nc.scalar.dma_start(out=sbs[s][:], in_=xw[s]).then_inc(sem_in[s], 16).ins
            for s in range(len(SLABS))
        ]

        # PE: consume slabs as they land; 2 matmuls (batch halves) per chunk
        # into the 2 PSUM banks, accumulating across all 9 chunks.
        last_mm0 = None
        for s, n in enumerate(SLABS):
            nc.tensor.wait_ge(sem_in[s], 16)
            xoff = n * B
            for j in range(n):
                c = SLAB_OFF[s] + j
                rhs = sbs[s][:, xoff + j * N_OUT:xoff + (j + 1) * N_OUT]
                mm0 = nc.tensor.matmul(
                    pts[0][:],
                    lhsT=sbs[s][:, j * B:j * B + MB],
                    rhs=rhs,
                    start=(c == 0),
                    stop=(c == KCH - 1),
                )
                nc.tensor.matmul(
                    pts[1][:],
                    lhsT=sbs[s][:, j * B + MB:j * B + 2 * MB],
                    rhs=rhs,
                    start=(c == 0),
                    stop=(c == KCH - 1),
                )
                last_mm0 = mm0
        last_mm0.then_inc(sem_d, 1)
        nc.tensor.drain().then_inc(sem_d, 1)

        # Tail: copy the two PSUM banks to SBUF (fp32 -> fp16) in parallel on
        # DVE (bank 0) and ACT (bank 1), then one f16 store from sync's ring.
        nc.vector.wait_ge(sem_d, 1)
        nc.vector.tensor_copy(osb[:, 0, :], pts[0][:]).then_inc(sem_cp, 1)
        nc.scalar.wait_ge(sem_d, 2)
        nc.scalar.copy(osb[:, 1, :], pts[1][:]).then_inc(sem_cp, 1)
        nc.sync.wait_ge(sem_cp, 2)
        nc.sync.dma_start(out=out, in_=osb[:]).then_inc(sem_out, 16)

        _hoist_first(nc, in_dmas)
        _strip_const_memsets(nc)
    nc.compile()
    return nc


def _prep_inputs(x, W):
    """Per-core host-side layout: packed per-slab [128, n*(256+160)] f16."""
    xs = np.ascontiguousarray(x[..., 0], dtype=np.float32)      # [B, P, 8]
    W = np.asarray(W, dtype=np.float32)
    in_maps = []
    for c in range(NCORES):
        pr = slice(c * PL, (c + 1) * PL)
        # x^T chunks: [128, KCH, B] with k_local = kc*128 + kp = p_local*8 + j
        xl = xs[:, pr, :].reshape(B, KL).T                      # [KL, B]
        xl = xl.reshape(KCH, 128, B).transpose(1, 0, 2)         # [128, KCH, B]
        # W2 chunks: W2[(p_local, j), (d, i)] = W[p, d, i, j]
        wl = W[pr].transpose(0, 3, 1, 2).reshape(KL, N_OUT)     # [KL, 160]
        wl = wl.reshape(KCH, 128, N_OUT).transpose(1, 0, 2)     # [128, KCH, 160]
        m = {}
        for s, n in enumerate(SLABS):
            o = SLAB_OFF[s]
            m[f"xw{s}"] = np.ascontiguousarray(np.concatenate([
                xl[:, o:o + n, :].reshape(128, n * B),
                wl[:, o:o + n, :].reshape(128, n * N_OUT),
            ], axis=1), dtype=np.float16)
        in_maps.append(m)
    return in_maps


def _squash(S):
    """S: [B, 160] summed partials -> squash over each group of 16."""
    S = S.reshape(B, D, VD)
    sq = np.sum(S * S, axis=2, keepdims=True)
    v = S * sq / (1.0 + sq) / np.sqrt(sq + 1e-9)
    return v[..., None].astype(np.float32)                      # [B, D, 16, 1]


def run(x, W, trace=False):
    if "nc" not in _cache:
        _cache["nc"] = _build()
    nc = _cache["nc"]
    in_maps = _prep_inputs(x, W)
    try:
        res = run_bass_kernel_spmd(nc, in_maps, core_ids=list(range(NCORES)), trace=trace)
    except Exception:
        # one retry absorbs transient runtime hiccups
        res = run_bass_kernel_spmd(nc, in_maps, core_ids=list(range(NCORES)), trace=trace)
    S = np.zeros((B, N_OUT), dtype=np.float32)
    for c in range(NCORES):
        # out[p, m, n] = S_partial[m*128+p, n]
        S += res.results[c]["out"].astype(np.float32).transpose(1, 0, 2).reshape(B, N_OUT)
    return _squash(S), res


def kernel(x, W):
    out, _ = run(np.asarray(x), np.asarray(W))
    return out
